# revision 14
# baseline (speedup 1.0000x reference)
"""ALIGNN layer on 8 Trainium2 NeuronCores (Bass/Tile).

Strategy:
- Edges balance-packed into 128-edge blocks per core (by line-edge count).
- Lines owned by their ld-edge's core, grouped per 4-block group, bucket-
  ordered by P1-slice (32768-row windows) for int16 dma_gather.
- line MLP1 factored: concat(ef[ls],ef[ld])@W1 = (ef@W1a)[ls] + (ef@W1b)[ld]
  -> dense P1/P2 per shard, AllGather P1, dma_gather per slice.
- h_pre transposed+accumulated on TensorE (P1.T + P2.T into PSUM), silu is
  the PSUM->SBUF move, mlp2 consumes feat-major hT as lhsT -> row-major y.
- scatter-mean = wide is_equal selector matmuls on TensorE + 1/cnt scale.
- Global attention computed transposed; softmax denom via ones-column in V.
"""

import math
import numpy as np

D = 128
H = 4
DH = 32
P = 128


def _snake_pack(weights, n_bins):
    n = len(weights)
    assert n % n_bins == 0
    order = np.argsort(-weights, kind="stable")
    bins = np.empty(n, dtype=np.int64)
    fwd = np.arange(n_bins)
    rev = fwd[::-1]
    for r in range(n // n_bins):
        sel = order[r * n_bins:(r + 1) * n_bins]
        bins[sel] = fwd if (r % 2 == 0) else rev
    return bins


def _wrap_idx(flat, cap):
    """[cap] -> [128, cap//16] int16 device layout (wrapped in 16 partitions,
    replicated for the 8 Q7 cores)."""
    assert cap % 16 == 0
    w = flat.reshape(cap // 16, 16).T.astype(np.int16)   # [16, cap//16]
    return np.tile(w, (8, 1))


def _prep(inputs, C, slice_rows=32768):
    nf = np.asarray(inputs["node_features"], np.float32)
    ef = np.asarray(inputs["edge_features"], np.float32)
    lef = np.asarray(inputs["line_edge_features"], np.float32)
    ei = np.asarray(inputs["edge_index"], np.int64)
    lei = np.asarray(inputs["line_edge_index"], np.int64)

    NN, _ = nf.shape
    E, _ = ef.shape
    L, _ = lef.shape
    EPC = E // C
    NPC = NN // C
    NBLK = EPC // P
    NNB = NPC // P
    assert E % (C * P) == 0 and NN % (C * P) == 0

    ls, ld = lei[0], lei[1]
    dst = ei[1]

    # ---- edge blocks (balanced by line count) ----
    cnt = np.bincount(ld, minlength=E).astype(np.int64)
    eblk = _snake_pack(cnt, C * NBLK)
    srt = np.argsort(eblk, kind="stable")
    pos_in_sorted = np.empty(E, dtype=np.int64)
    pos_in_sorted[srt] = np.arange(E)
    blk_start = np.searchsorted(eblk[srt], np.arange(C * NBLK))
    eslot = pos_in_sorted - blk_start[eblk]
    perm_pos = eblk * P + eslot
    edge_at = np.empty(E, dtype=np.int64)
    edge_at[perm_pos] = np.arange(E)

    # slices of the permuted edge table (for int16 gathers)
    NSLC = int(math.ceil(E / slice_rows))
    SLICE = int(math.ceil(E / NSLC / P)) * P
    NSLC = int(math.ceil(E / SLICE))
    assert SLICE <= 32768

    G = 4 if NBLK % 4 == 0 else (2 if NBLK % 2 == 0 else 1)
    NGRP = NBLK // G

    # ---- assign lines to (core, group, slice-bucket) ----
    lgrp = eblk[ld] // G                        # global group id of each line
    lslc = perm_pos[ls] // SLICE                # src slice of P1 row
    key = lgrp * NSLC + lslc
    lsrt = np.argsort(key, kind="stable")
    kb_start = np.searchsorted(key[lsrt], np.arange(C * NGRP * NSLC + 1))
    bucket_sizes = np.diff(kb_start)
    CAP = int(math.ceil(max(bucket_sizes.max(), 1) / P)) * P
    CAPC = CAP // P                             # chunks per bucket
    NCH = NSLC * CAPC                           # chunks per group
    LPG = NCH * P                               # line slots per group
    LPC = NGRP * LPG

    # ---- node blocks ----
    indeg = np.bincount(dst, minlength=NN).astype(np.int64)
    nblk = _snake_pack(indeg, C * NNB)
    nsrt0 = np.argsort(nblk, kind="stable")
    npos = np.empty(NN, dtype=np.int64)
    npos[nsrt0] = np.arange(NN)
    nb_start = np.searchsorted(nblk[nsrt0], np.arange(C * NNB))
    nslot = npos - nb_start[nblk]
    nperm_pos = nblk * P + nslot
    node_at = np.empty(NN, dtype=np.int64)
    node_at[nperm_pos] = np.arange(NN)

    # ---- edges grouped by (dst node block, slice of eh row) ----
    eslc = perm_pos // SLICE
    nkey = nblk[dst] * NSLC + eslc
    dsrt = np.argsort(nkey, kind="stable")
    nkb_start = np.searchsorted(nkey[dsrt], np.arange(C * NNB * NSLC + 1))
    nbucket = np.diff(nkb_start)
    CAPN = int(math.ceil(max(nbucket.max(), 1) / P)) * P
    CAPNC = CAPN // P
    KCNE = NSLC * CAPNC                         # chunks per node block

    st = dict(C=C, NN=NN, E=E, L=L, EPC=EPC, NPC=NPC, NBLK=NBLK, NNB=NNB,
              G=G, NGRP=NGRP, NSLC=NSLC, SLICE=SLICE, CAP=CAP, CAPC=CAPC,
              NCH=NCH, LPG=LPG, LPC=LPC, CAPN=CAPN, CAPNC=CAPNC, KCNE=KCNE)

    cores = []
    for c in range(C):
        # ----- line slots -----
        lid = np.full(LPC, -1, dtype=np.int64)
        ip1 = np.zeros((NGRP, NSLC, CAP), np.int64)
        for g in range(NGRP):
            for s in range(NSLC):
                kb = (c * NGRP + g) * NSLC + s
                ids = lsrt[kb_start[kb]:kb_start[kb + 1]]
                base = g * LPG + s * CAP
                lid[base:base + len(ids)] = ids
                ip1[g, s, :len(ids)] = perm_pos[ls[ids]] - s * SLICE
        pad = lid < 0
        lid0 = np.where(pad, 0, lid)
        lef_s = lef[lid0].copy()
        lef_s[pad] = 0.0
        ip2 = (perm_pos[ld[lid0]] - c * EPC)
        ip2[pad] = 0
        ldc = (eblk[ld[lid0]] % G) * P + eslot[ld[lid0]]
        ldc = ldc.astype(np.float32)
        ldc[pad] = -1.0

        ip1_d = np.zeros((NGRP, NSLC, P, CAP // 16), np.int16)
        for g in range(NGRP):
            for s in range(NSLC):
                ip1_d[g, s] = _wrap_idx(ip1[g, s], CAP)
        ip2_d = np.zeros((NGRP, P, LPG // 16), np.int16)
        for g in range(NGRP):
            ip2_d[g] = _wrap_idx(ip2[g * LPG:(g + 1) * LPG], LPG)
        ldc_d = np.ascontiguousarray(
            ldc.reshape(NGRP, NCH, P).transpose(0, 2, 1))     # [NGRP,128,NCH]

        cnt_c = cnt[edge_at[c * EPC:(c + 1) * EPC]].reshape(NBLK, P)
        invc = np.ascontiguousarray(
            (1.0 / np.clip(cnt_c, 1, None)).astype(np.float32).T)
        ef_s = ef[edge_at[c * EPC:(c + 1) * EPC]].copy()
        nf_s = nf[node_at[c * NPC:(c + 1) * NPC]].copy()

        # ----- node-stage edge slots -----
        eid_n = np.full(NNB * KCNE * P, -1, dtype=np.int64)
        iehs = np.zeros((NNB, NSLC, CAPN), np.int64)
        for nb in range(NNB):
            for s in range(NSLC):
                kb = (c * NNB + nb) * NSLC + s
                ids = dsrt[nkb_start[kb]:nkb_start[kb + 1]]
                base = nb * KCNE * P + s * CAPN
                eid_n[base:base + len(ids)] = ids
                iehs[nb, s, :len(ids)] = perm_pos[ids] - s * SLICE
        npad = eid_n < 0
        eid_n0 = np.where(npad, 0, eid_n)
        dstc = nslot[dst[eid_n0]].astype(np.float32)
        dstc[npad] = -1.0
        ieh_d = np.zeros((NNB, NSLC, P, CAPN // 16), np.int16)
        for nb in range(NNB):
            for s in range(NSLC):
                ieh_d[nb, s] = _wrap_idx(iehs[nb, s], CAPN)
        dstc_d = np.ascontiguousarray(
            dstc.reshape(NNB, KCNE, P).transpose(0, 2, 1))    # [NNB,128,KCNE]
        ind_c = indeg[node_at[c * NPC:(c + 1) * NPC]].reshape(NNB, P)
        invd = np.ascontiguousarray(
            (1.0 / np.clip(ind_c, 1, None)).astype(np.float32).T)

        cores.append(dict(
            lef_s=lef_s, ef_s=ef_s, nf_s=nf_s,
            ip1=ip1_d, ip2=ip2_d, ldc=ldc_d, invc=invc,
            ieh=ieh_d, dstc=dstc_d, invd=invd,
            lid=lid, eids=edge_at[c * EPC:(c + 1) * EPC],
            nids=node_at[c * NPC:(c + 1) * NPC],
        ))
    return st, cores


def _fold_weights(inputs, G):
    g = lambda k: np.asarray(inputs[k], np.float32)
    w = {}
    le_w1 = g("le_w1")
    w["w1cat"] = np.ascontiguousarray(
        np.concatenate([le_w1[:D], le_w1[D:]], axis=1))      # [128,256]
    w["lw2"] = g("le_w2")
    e_w1 = g("e_w1")
    w["ew1ef"] = np.ascontiguousarray(e_w1[:D] + e_w1[D:2 * D])   # [128,256]
    w["ew1la"] = np.ascontiguousarray(e_w1[2 * D:])
    w["ew2"] = np.ascontiguousarray(
        g("e_w2").reshape(2, D, D).transpose(1, 0, 2))        # [128,2,128]
    n_w1 = g("n_w1")
    w["nw1"] = np.ascontiguousarray(
        n_w1.reshape(2, D, 2 * D).transpose(1, 0, 2))         # [128,2,256]
    w["nw2"] = np.ascontiguousarray(
        g("n_w2").reshape(2, D, D).transpose(1, 0, 2))
    wqkv = g("wqkv")
    w["wq"] = np.ascontiguousarray(wqkv[:, :D] / math.sqrt(DH))
    w["wk"] = np.ascontiguousarray(wqkv[:, D:2 * D])
    w["wv"] = np.ascontiguousarray(wqkv[:, 2 * D:])
    w["wo"] = g("wo")
    w["iota"] = np.ascontiguousarray(
        np.broadcast_to(np.arange(G * P, dtype=np.float32), (P, G * P)))

    flags = {}
    def nz(k):
        return not np.allclose(g(k), 0.0)
    def row_tile(k):
        v = g(k)
        return np.ascontiguousarray(
            np.broadcast_to(v[None, :], (P, v.shape[0])).astype(np.float32))
    for k in ["le_b1", "le_b2", "e_b1", "e_b2", "n_b1", "n_b2", "bqkv", "bo"]:
        flags[k] = nz(k)
    if flags["le_b1"]:
        w["lb1_col"] = np.ascontiguousarray(g("le_b1").reshape(D, 1))
    if flags["le_b2"]:
        w["bt_le_b2"] = row_tile("le_b2")
    if flags["e_b1"]:
        w["eb1_col"] = np.ascontiguousarray(g("e_b1").reshape(2, D).T.copy())
    if flags["e_b2"]:
        w["bt_e_b2"] = row_tile("e_b2")
    if flags["n_b1"]:
        w["nb1_col"] = np.ascontiguousarray(g("n_b1").reshape(2, D).T.copy())
    if flags["n_b2"]:
        w["bt_n_b2"] = row_tile("n_b2")
    if flags["bo"]:
        w["bt_bo"] = row_tile("bo")
    if flags["bqkv"]:
        bq = g("bqkv")
        w["bq_col"] = np.ascontiguousarray(
            (bq[:D] / math.sqrt(DH)).reshape(D, 1))
        w["bk_col"] = np.ascontiguousarray(bq[D:2 * D].reshape(D, 1))
        w["bv_row"] = np.ascontiguousarray(
            np.broadcast_to(bq[None, 2 * D:], (P, D)).astype(np.float32))
    lngs = {}
    for nm in ["node", "edge", "line"]:
        gv, bv = g(f"ln_{nm}_g"), g(f"ln_{nm}_b")
        triv = np.allclose(gv, 1.0) and np.allclose(bv, 0.0)
        lngs[nm] = not triv
        if not triv:
            w[f"lng_{nm}"] = np.ascontiguousarray(
                np.broadcast_to(gv[None, :], (P, D)).astype(np.float32))
            w[f"lnb_{nm}"] = np.ascontiguousarray(
                np.broadcast_to(bv[None, :], (P, D)).astype(np.float32))
    flags["ln"] = lngs
    return w, flags


# ---------------------------------------------------------------------------
# bass builder
# ---------------------------------------------------------------------------

def _build(st, wflags, silu_native=True):
    import concourse.bass as bass
    import concourse.bacc as bacc
    import concourse.tile as tile
    from concourse import mybir
    from concourse.masks import make_identity
    from contextlib import ExitStack

    w, flags = wflags
    f32 = mybir.dt.float32
    i16 = mybir.dt.int16
    AF = mybir.ActivationFunctionType
    ALU = mybir.AluOpType

    C, EPC, NPC = st["C"], st["EPC"], st["NPC"]
    NBLK, NNB = st["NBLK"], st["NNB"]
    G, NGRP, NSLC, SLICE = st["G"], st["NGRP"], st["NSLC"], st["SLICE"]
    CAP, CAPC, NCH, LPG, LPC = (st["CAP"], st["CAPC"], st["NCH"],
                                st["LPG"], st["LPC"])
    CAPN, CAPNC, KCNE = st["CAPN"], st["CAPNC"], st["KCNE"]
    NN = st["NN"]
    NTM = NN // P
    RG = [list(range(C))]
    E_PERM = NBLK * P * C

    nc = bacc.Bacc(None, num_devices=C)

    def dp(name, shape, dt=f32, out=False):
        return nc.declare_dram_parameter(name, list(shape), dt, isOutput=out)

    t_lef = dp("lef_s", (LPC, D))
    t_ef = dp("ef_s", (EPC, D))
    t_nf = dp("nf_s", (NPC, D))
    t_ip1 = dp("ip1", (NGRP, NSLC, P, CAP // 16), i16)
    t_ip2 = dp("ip2", (NGRP, P, LPG // 16), i16)
    t_ldc = dp("ldc", (NGRP, P, NCH))
    t_invc = dp("invc", (P, NBLK))
    t_ieh = dp("ieh", (NNB, NSLC, P, CAPN // 16), i16)
    t_dstc = dp("dstc", (NNB, P, KCNE))
    t_invd = dp("invd", (P, NNB))
    wt = {}
    for k, v in w.items():
        wt[k] = dp("w_" + k, v.shape)
    o_line = dp("out_line", (LPC, D), out=True)
    o_edge = dp("out_edge", (EPC, D), out=True)
    o_node = dp("out_node", (NPC, D), out=True)

    with tile.TileContext(nc) as tc, ExitStack() as ctx:
        dram = ctx.enter_context(tc.tile_pool(name="dram", bufs=1, space="DRAM"))
        SHR = "Shared" if C > 4 else "Local"
        p1_loc = dram.tile([EPC, D], f32)
        p2_loc = dram.tile([EPC, D], f32)
        p1_full = dram.tile([E_PERM, D], f32, addr_space=SHR)
        eh_loc = dram.tile([EPC, D], f32)
        eh_full = dram.tile([E_PERM, D], f32, addr_space=SHR)
        kt_loc = dram.tile([P, NPC], f32)
        kt_full = dram.tile([C, P, NPC], f32, addr_space=SHR)
        v_loc = dram.tile([NPC, D], f32)
        v_full = dram.tile([NN, D], f32, addr_space=SHR)

        cst = ctx.enter_context(tc.tile_pool(name="cst", bufs=1))
        ident = cst.tile([P, P], f32)
        make_identity(nc, ident[:])
        iota_t = cst.tile([P, G * P], f32)
        nc.sync.dma_start(out=iota_t[:], in_=wt["iota"][:, :])
        eps_t = cst.tile([P, 1], f32)
        nc.vector.memset(eps_t[:], 1e-5)
        ones_row = cst.tile([1, 32], f32)
        nc.vector.memset(ones_row[:], 1.0)

        wsb = {}
        for k, v in w.items():
            if k == "iota":
                continue
            wsb[k] = cst.tile(list(v.shape), f32, name="w_" + k)
            nc.sync.dma_start(out=wsb[k][:], in_=wt[k][:])
        invc_sb = cst.tile([P, NBLK], f32)
        nc.sync.dma_start(out=invc_sb[:], in_=t_invc[:, :])
        invd_sb = cst.tile([P, NNB], f32)
        nc.sync.dma_start(out=invd_sb[:], in_=t_invd[:, :])

        efT = cst.tile([P, EPC], f32)

        def silu_op(out_ap, in_ap, tmp_pool, shape, tag, bias=None):
            if silu_native:
                if bias is None:
                    nc.scalar.activation(out=out_ap, in_=in_ap, func=AF.Silu)
                else:
                    nc.scalar.activation(out=out_ap, in_=in_ap, func=AF.Silu,
                                         bias=bias, scale=1.0)
            else:
                src = in_ap
                if bias is not None:
                    b = tmp_pool.tile(shape, f32, tag=tag + "b",
                                      name="sb_" + tag)
                    nc.vector.tensor_scalar(out=b[:], in0=in_ap, scalar1=bias,
                                            scalar2=None, op0=ALU.add)
                    src = b[:]
                tmp = tmp_pool.tile(shape, f32, tag=tag, name="silu_" + tag)
                nc.scalar.activation(out=tmp[:], in_=src, func=AF.Sigmoid)
                nc.vector.tensor_tensor(out=out_ap, in0=src, in1=tmp[:],
                                        op=ALU.mult)

        dve_copy = nc.vector.tensor_copy
        act_copy = nc.scalar.copy

        def tpose(pool_ps, dst_ap, src_ap, copy_eng, tag="tp"):
            ps = pool_ps.tile([P, P], f32, tag=tag)
            nc.tensor.transpose(out=ps[:], in_=src_ap, identity=ident[:])
            copy_eng(out=dst_ap, in_=ps[:])

        # =============== phase D: dense P1/P2 + efT ===============
        with tc.tile_pool(name="dph", bufs=3) as dph, \
             tc.tile_pool(name="dps", bufs=2, space="PSUM") as dps:
            for g in range(NBLK // G):
                ef_t = dph.tile([P, G, D], f32, tag="ef")
                nc.sync.dma_start(
                    out=ef_t[:],
                    in_=t_ef[g * G * P:(g + 1) * G * P, :].rearrange(
                        "(q p) f -> p q f", p=P))
                pq_sb = dph.tile([P, G, 2 * D], f32, tag="pq")
                for t in range(G):
                    b = g * G + t
                    tpose(dps, efT[:, b * P:(b + 1) * P], ef_t[:, t, :], dve_copy)
                    ps = dps.tile([P, 2 * D], f32, tag="mm")
                    nc.tensor.matmul(out=ps[:], lhsT=efT[:, b * P:(b + 1) * P],
                                     rhs=wsb["w1cat"][:], start=True, stop=True)
                    act_copy(out=pq_sb[:, t, :], in_=ps[:])
                nc.sync.dma_start(
                    out=p1_loc[g * G * P:(g + 1) * G * P, :].rearrange(
                        "(q p) f -> p q f", p=P),
                    in_=pq_sb[:, :, 0:D])
                nc.sync.dma_start(
                    out=p2_loc[g * G * P:(g + 1) * G * P, :].rearrange(
                        "(q p) f -> p q f", p=P),
                    in_=pq_sb[:, :, D:2 * D])

        nc.gpsimd.collective_compute(
            "AllGather", ALU.bypass, replica_groups=RG,
            ins=[p1_loc[:, :].opt()], outs=[p1_full[:, :].opt()])

        # =============== line + edge stage ===============
        SUB = 6 if NCH % 6 == 0 else (4 if NCH % 4 == 0 else
                                      (3 if NCH % 3 == 0 else
                                       (2 if NCH % 2 == 0 else 1)))
        NSUB = NCH // SUB
        with tc.tile_pool(name="lp", bufs=2) as lp, \
             tc.tile_pool(name="lp3", bufs=3) as lp3, \
             tc.tile_pool(name="lps", bufs=2, space="PSUM") as lps, \
             tc.tile_pool(name="lps1", bufs=1, space="PSUM") as lps1:
            for g in range(NGRP):
                idx1 = lp3.tile([P, NSLC, CAP // 16], i16, tag="i1")
                nc.sync.dma_start(out=idx1[:], in_=t_ip1[g].rearrange(
                    "s p c -> p s c"))
                idx2 = lp3.tile([P, LPG // 16], i16, tag="i2")
                nc.sync.dma_start(out=idx2[:], in_=t_ip2[g, :, :])
                ldc = lp.tile([P, NCH], f32, tag="ldc")
                nc.sync.dma_start(out=ldc[:], in_=t_ldc[g, :, :])
                lef_t = lp.tile([P, NCH, D], f32, tag="lef")
                nc.sync.dma_start(
                    out=lef_t[:],
                    in_=t_lef[g * LPG:(g + 1) * LPG, :].rearrange(
                        "(q p) f -> p q f", p=P))
                ef_row = lp.tile([P, G, D], f32, tag="efr")
                nc.sync.dma_start(
                    out=ef_row[:],
                    in_=t_ef[g * G * P:(g + 1) * G * P, :].rearrange(
                        "(q p) f -> p q f", p=P))

                hp1 = lp.tile([P, NCH, D], f32, tag="hp1")
                for s in range(NSLC):
                    nc.gpsimd.dma_gather(
                        out_ap=hp1[:, s * CAPC:(s + 1) * CAPC, :],
                        in_ap=p1_full[s * SLICE:min((s + 1) * SLICE, E_PERM), :],
                        idxs_ap=idx1[:, s, :], num_idxs=CAP, num_idxs_reg=CAP,
                        elem_size=D, single_packet=False)
                hp2 = lp.tile([P, NCH, D], f32, tag="hp2")
                nc.gpsimd.dma_gather(
                    out_ap=hp2[:], in_ap=p2_loc[:, :], idxs_ap=idx2[:],
                    num_idxs=LPG, num_idxs_reg=LPG, elem_size=D,
                    single_packet=False)

                vh = lp.tile([P, NCH, D], f32, tag="vh")
                yb = lp.tile([P, NCH, D], f32, tag="yb")
                mvs = lp.tile([P, NCH, 2], f32, tag="mvs")
                agg = lps1.tile([P, G, P], f32, tag="agg")
                eh_sb = lp.tile([P, G, D], f32, tag="ehsb")
                emv = lp.tile([P, G, 2], f32, tag="emv")
                for sub in range(NSUB):
                    for k in range(SUB):
                        q = sub * SUB + k
                        hT = lps.tile([P, P], f32, tag="tp")
                        nc.tensor.matmul(out=hT[:], lhsT=hp1[:, q, :],
                                         rhs=ident[:], is_transpose=True,
                                         start=True, stop=False)
                        nc.tensor.matmul(out=hT[:], lhsT=hp2[:, q, :],
                                         rhs=ident[:], is_transpose=True,
                                         start=False, stop=True)
                        hT_sb = lp3.tile([P, P], f32, tag="hts")
                        silu_op(hT_sb[:], hT[:], lp3, [P, P], "sl1",
                                bias=(wsb["lb1_col"][:] if flags["le_b1"]
                                      else None))
                        yk = lps.tile([P, P], f32, tag="yk")
                        nc.tensor.matmul(out=yk[:], lhsT=hT_sb[:],
                                         rhs=wsb["lw2"][:], start=True,
                                         stop=False)
                        nc.tensor.matmul(out=yk[:], lhsT=ident[:],
                                         rhs=lef_t[:, q, :], start=False,
                                         stop=True)
                        act_copy(out=yb[:, q, :], in_=yk[:])
                        if flags["le_b2"]:
                            nc.vector.tensor_tensor(
                                out=yb[:, q, :], in0=yb[:, q, :],
                                in1=wsb["bt_le_b2"][:], op=ALU.add)
                        stt = lp3.tile([P, 6], f32, tag="stt")
                        nc.vector.bn_stats(out=stt[:], in_=yb[:, q, :])
                        nc.vector.bn_aggr(out=mvs[:, q, :], in_=stt[:])
                    rstd = lp3.tile([P, SUB], f32, tag="rstd")
                    nc.scalar.activation(
                        out=rstd[:], in_=mvs[:, sub * SUB:(sub + 1) * SUB, 1],
                        func=AF.Sqrt, bias=eps_t[:], scale=1.0)
                    nc.vector.reciprocal(out=rstd[:], in_=rstd[:])
                    for k in range(SUB):
                        q = sub * SUB + k
                        nc.vector.tensor_scalar(
                            out=vh[:, q, :], in0=yb[:, q, :],
                            scalar1=mvs[:, q, 0:1], scalar2=rstd[:, k:k + 1],
                            op0=ALU.subtract, op1=ALU.mult)
                        if flags["ln"]["line"]:
                            nc.vector.tensor_tensor(
                                out=vh[:, q, :], in0=vh[:, q, :],
                                in1=wsb["lng_line"][:], op=ALU.mult)
                            nc.vector.tensor_tensor(
                                out=vh[:, q, :], in0=vh[:, q, :],
                                in1=wsb["lnb_line"][:], op=ALU.add)
                        sel = lp3.tile([P, G * P], f32, tag="sel")
                        nc.vector.tensor_scalar(
                            out=sel[:], in0=iota_t[:], scalar1=ldc[:, q:q + 1],
                            scalar2=None, op0=ALU.is_equal)
                        for b in range(G):
                            nc.tensor.matmul(
                                out=agg[:, b, :],
                                lhsT=sel[:, b * P:(b + 1) * P],
                                rhs=vh[:, q, :],
                                start=(q == 0 and b == 0),
                                stop=(q == NCH - 1 and b == G - 1))
                # write V-hat
                nc.sync.dma_start(
                    out=o_line[g * LPG:(g + 1) * LPG, :].rearrange(
                        "(q p) f -> p q f", p=P),
                    in_=vh[:])
                # ---- edge update for the G blocks ----
                for t in range(G):
                    b = g * G + t
                    la = lp3.tile([P, D], f32, tag="la")
                    nc.vector.tensor_scalar(
                        out=la[:], in0=agg[:, t, :], scalar1=invc_sb[:, b:b + 1],
                        scalar2=None, op0=ALU.mult)
                    laT = lp3.tile([P, D], f32, tag="laT")
                    tpose(lps, laT[:], la[:], dve_copy)
                    heT = lps.tile([P, 2, D], f32, tag="heT")
                    for hh in range(2):
                        nc.tensor.matmul(
                            out=heT[:, hh, :],
                            lhsT=wsb["ew1ef"][:, hh * P:(hh + 1) * P],
                            rhs=efT[:, b * P:(b + 1) * P],
                            start=(hh == 0), stop=False)
                        nc.tensor.matmul(
                            out=heT[:, hh, :],
                            lhsT=wsb["ew1la"][:, hh * P:(hh + 1) * P],
                            rhs=laT[:], start=False, stop=(hh == 1))
                    heT_sb = lp3.tile([P, 2, D], f32, tag="hesb")
                    for hh in range(2):
                        silu_op(heT_sb[:, hh, :], heT[:, hh, :], lp3,
                                [P, D], "sl2",
                                bias=(wsb["eb1_col"][:, hh:hh + 1]
                                      if flags["e_b1"] else None))
                    ye = lps.tile([P, D], f32, tag="yk")
                    nc.tensor.matmul(out=ye[:], lhsT=heT_sb[:, 0, :],
                                     rhs=wsb["ew2"][:, 0, :], start=True,
                                     stop=False)
                    nc.tensor.matmul(out=ye[:], lhsT=heT_sb[:, 1, :],
                                     rhs=wsb["ew2"][:, 1, :], start=False,
                                     stop=False)
                    nc.tensor.matmul(out=ye[:], lhsT=ident[:],
                                     rhs=ef_row[:, t, :], start=False,
                                     stop=True)
                    act_copy(out=eh_sb[:, t, :], in_=ye[:])
                    if flags["e_b2"]:
                        nc.vector.tensor_tensor(
                            out=eh_sb[:, t, :], in0=eh_sb[:, t, :],
                            in1=wsb["bt_e_b2"][:], op=ALU.add)
                    stt2 = lp3.tile([P, 6], f32, tag="stt2")
                    nc.vector.bn_stats(out=stt2[:], in_=eh_sb[:, t, :])
                    nc.vector.bn_aggr(out=emv[:, t, :], in_=stt2[:])
                erstd = lp3.tile([P, G], f32, tag="erstd")
                nc.scalar.activation(out=erstd[:], in_=emv[:, :, 1],
                                     func=AF.Sqrt, bias=eps_t[:], scale=1.0)
                nc.vector.reciprocal(out=erstd[:], in_=erstd[:])
                ehn = lp.tile([P, G, D], f32, tag="ehn")
                for t in range(G):
                    nc.vector.tensor_scalar(
                        out=ehn[:, t, :], in0=eh_sb[:, t, :],
                        scalar1=emv[:, t, 0:1], scalar2=erstd[:, t:t + 1],
                        op0=ALU.subtract, op1=ALU.mult)
                    if flags["ln"]["edge"]:
                        nc.vector.tensor_tensor(
                            out=ehn[:, t, :], in0=ehn[:, t, :],
                            in1=wsb["lng_edge"][:], op=ALU.mult)
                        nc.vector.tensor_tensor(
                            out=ehn[:, t, :], in0=ehn[:, t, :],
                            in1=wsb["lnb_edge"][:], op=ALU.add)
                nc.sync.dma_start(
                    out=o_edge[g * G * P:(g + 1) * G * P, :].rearrange(
                        "(q p) f -> p q f", p=P),
                    in_=ehn[:])
                nc.sync.dma_start(
                    out=eh_loc[g * G * P:(g + 1) * G * P, :].rearrange(
                        "(q p) f -> p q f", p=P),
                    in_=ehn[:])

        nc.gpsimd.collective_compute(
            "AllGather", ALU.bypass, replica_groups=RG,
            ins=[eh_loc[:, :].opt()], outs=[eh_full[:, :].opt()])

        # =============== node stage ===============
        xh = cst.tile([P, NNB, D], f32)
        with tc.tile_pool(name="np2", bufs=2) as np2, \
             tc.tile_pool(name="nps", bufs=2, space="PSUM") as nps, \
             tc.tile_pool(name="nps1", bufs=1, space="PSUM") as nps1:
            for nb in range(NNB):
                idxn = np2.tile([P, NSLC, CAPN // 16], i16, tag="ixn")
                nc.sync.dma_start(out=idxn[:], in_=t_ieh[nb].rearrange(
                    "s p c -> p s c"))
                dstc = np2.tile([P, KCNE], f32, tag="dstc")
                nc.sync.dma_start(out=dstc[:], in_=t_dstc[nb, :, :])
                gath = np2.tile([P, KCNE, D], f32, tag="gath")
                for s in range(NSLC):
                    nc.gpsimd.dma_gather(
                        out_ap=gath[:, s * CAPNC:(s + 1) * CAPNC, :],
                        in_ap=eh_full[s * SLICE:min((s + 1) * SLICE, E_PERM), :],
                        idxs_ap=idxn[:, s, :], num_idxs=CAPN, num_idxs_reg=CAPN,
                        elem_size=D, single_packet=False)
                nagg = nps1.tile([P, D], f32, tag="nagg")
                for k in range(KCNE):
                    sel = np2.tile([P, P], f32, tag="nsel")
                    nc.vector.tensor_scalar(
                        out=sel[:], in0=iota_t[:, 0:P], scalar1=dstc[:, k:k + 1],
                        scalar2=None, op0=ALU.is_equal)
                    nc.tensor.matmul(out=nagg[:], lhsT=sel[:], rhs=gath[:, k, :],
                                     start=(k == 0), stop=(k == KCNE - 1))
                na = np2.tile([P, D], f32, tag="na")
                nc.vector.tensor_scalar(
                    out=na[:], in0=nagg[:], scalar1=invd_sb[:, nb:nb + 1],
                    scalar2=None, op0=ALU.mult)
                nfr = np2.tile([P, D], f32, tag="nfr")
                nc.sync.dma_start(out=nfr[:], in_=t_nf[nb * P:(nb + 1) * P, :])
                nfT = np2.tile([P, D], f32, tag="nfT")
                tpose(nps, nfT[:], nfr[:], dve_copy)
                naT = np2.tile([P, D], f32, tag="naT")
                tpose(nps, naT[:], na[:], dve_copy)
                hnT = nps1.tile([P, 2, D], f32, tag="hnT")
                for hh in range(2):
                    nc.tensor.matmul(
                        out=hnT[:, hh, :],
                        lhsT=wsb["nw1"][:, 0, hh * P:(hh + 1) * P],
                        rhs=nfT[:], start=(hh == 0), stop=False)
                    nc.tensor.matmul(
                        out=hnT[:, hh, :],
                        lhsT=wsb["nw1"][:, 1, hh * P:(hh + 1) * P],
                        rhs=naT[:], start=False, stop=(hh == 1))
                hnT_sb = np2.tile([P, 2, D], f32, tag="hnsb")
                for hh in range(2):
                    silu_op(hnT_sb[:, hh, :], hnT[:, hh, :], np2, [P, D], "sl3",
                            bias=(wsb["nb1_col"][:, hh:hh + 1]
                                  if flags["n_b1"] else None))
                yn = nps.tile([P, D], f32, tag="yn")
                nc.tensor.matmul(out=yn[:], lhsT=hnT_sb[:, 0, :],
                                 rhs=wsb["nw2"][:, 0, :], start=True, stop=False)
                nc.tensor.matmul(out=yn[:], lhsT=hnT_sb[:, 1, :],
                                 rhs=wsb["nw2"][:, 1, :], start=False, stop=False)
                nc.tensor.matmul(out=yn[:], lhsT=ident[:], rhs=nfr[:],
                                 start=False, stop=True)
                yn_sb = np2.tile([P, D], f32, tag="ynsb")
                act_copy(out=yn_sb[:], in_=yn[:])
                if flags["n_b2"]:
                    nc.vector.tensor_tensor(out=yn_sb[:], in0=yn_sb[:],
                                            in1=wsb["bt_n_b2"][:], op=ALU.add)
                stt3 = np2.tile([P, 6], f32, tag="stt3")
                nc.vector.bn_stats(out=stt3[:], in_=yn_sb[:])
                nmv = np2.tile([P, 2], f32, tag="nmv")
                nc.vector.bn_aggr(out=nmv[:], in_=stt3[:])
                nrstd = np2.tile([P, 1], f32, tag="nrstd")
                nc.scalar.activation(out=nrstd[:], in_=nmv[:, 1:2],
                                     func=AF.Sqrt, bias=eps_t[:], scale=1.0)
                nc.vector.reciprocal(out=nrstd[:], in_=nrstd[:])
                nc.vector.tensor_scalar(
                    out=xh[:, nb, :], in0=yn_sb[:],
                    scalar1=nmv[:, 0:1], scalar2=nrstd[:],
                    op0=ALU.subtract, op1=ALU.mult)
                if flags["ln"]["node"]:
                    nc.vector.tensor_tensor(out=xh[:, nb, :], in0=xh[:, nb, :],
                                            in1=wsb["lng_node"][:], op=ALU.mult)
                    nc.vector.tensor_tensor(out=xh[:, nb, :], in0=xh[:, nb, :],
                                            in1=wsb["lnb_node"][:], op=ALU.add)

        # =============== attention ===============
        with tc.tile_pool(name="ap1", bufs=1) as ap1, \
             tc.tile_pool(name="ap2", bufs=2) as ap2, \
             tc.tile_pool(name="aps", bufs=1, space="PSUM") as aps, \
             tc.tile_pool(name="aps4", bufs=4, space="PSUM") as aps4:
            xT = ap1.tile([P, NPC], f32)
            for nb in range(NNB):
                tpose(aps4, xT[:, nb * P:(nb + 1) * P], xh[:, nb, :], dve_copy,
                      tag="scr")
            qT = ap1.tile([P, NPC], f32)
            kT = ap1.tile([P, NPC], f32)
            qps = aps4.tile([P, NPC], f32, tag="scr")
            nc.tensor.matmul(out=qps[:], lhsT=wsb["wq"][:], rhs=xT[:],
                             start=True, stop=True)
            if flags["bqkv"]:
                nc.vector.tensor_scalar(out=qT[:], in0=qps[:],
                                        scalar1=wsb["bq_col"][:],
                                        scalar2=None, op0=ALU.add)
            else:
                dve_copy(out=qT[:], in_=qps[:])
            kps = aps4.tile([P, NPC], f32, tag="scr")
            nc.tensor.matmul(out=kps[:], lhsT=wsb["wk"][:], rhs=xT[:],
                             start=True, stop=True)
            if flags["bqkv"]:
                nc.vector.tensor_scalar(out=kT[:], in0=kps[:],
                                        scalar1=wsb["bk_col"][:],
                                        scalar2=None, op0=ALU.add)
            else:
                dve_copy(out=kT[:], in_=kps[:])
            nc.sync.dma_start(out=kt_loc[:, :], in_=kT[:])
            vrow = ap1.tile([P, NNB, D], f32)
            for nb in range(NNB):
                vps = aps4.tile([P, NPC], f32, tag="scr")
                nc.tensor.matmul(out=vps[:, 0:D],
                                 lhsT=xT[:, nb * P:(nb + 1) * P],
                                 rhs=wsb["wv"][:], start=True, stop=True)
                if flags["bqkv"]:
                    nc.vector.tensor_tensor(out=vrow[:, nb, :], in0=vps[:, 0:D],
                                            in1=wsb["bv_row"][:], op=ALU.add)
                else:
                    act_copy(out=vrow[:, nb, :], in_=vps[:, 0:D])
            nc.sync.dma_start(out=v_loc[:, :].rearrange("(q p) f -> p q f", p=P),
                              in_=vrow[:])
            nc.gpsimd.collective_compute(
                "AllGather", ALU.bypass, replica_groups=RG,
                ins=[kt_loc[:, :].opt()], outs=[kt_full[:, :, :].opt()])
            nc.gpsimd.collective_compute(
                "AllGather", ALU.bypass, replica_groups=RG,
                ins=[v_loc[:, :].opt()], outs=[v_full[:, :].opt()])

            KTs = ap1.tile([P, C, NPC], f32)
            nc.sync.dma_start(out=KTs[:], in_=kt_full[:, :, :].rearrange(
                "c d n -> d c n"))
            vsb = ap1.tile([P, NTM, H, 34], f32)
            nc.vector.memset(vsb[:], 0.0)
            v4 = v_full[:, :].rearrange("(j p) (h u) -> p j h u", p=P, h=H)
            for h in range(H):
                nc.sync.dma_start(out=vsb[:, :, h, 0:32], in_=v4[:, :, h, :])
            nc.vector.memset(vsb[:, :, :, 32:33], 1.0)

            KTf = KTs[:].rearrange("d c n -> d (c n)")
            ctx_ps = [aps.tile([33, NPC], f32, name=f"ctx{h}") for h in range(H)]
            for j in range(NTM):
                es = ap2.tile([P, H, NPC], f32, tag="es")
                for h in range(H):
                    sps = aps4.tile([P, NPC], f32, tag="scr")
                    nc.tensor.matmul(
                        out=sps[:],
                        lhsT=KTf[DH * h:DH * (h + 1), j * P:(j + 1) * P],
                        rhs=qT[DH * h:DH * (h + 1), :],
                        start=True, stop=True, tile_position=(DH * h, 0))
                    nc.scalar.activation(out=es[:, h, :], in_=sps[:], func=AF.Exp)
                for h in range(H):
                    nc.tensor.matmul(
                        out=ctx_ps[h][:], lhsT=vsb[:, j, h, 0:33],
                        rhs=es[:, h, :], start=(j == 0), stop=(j == NTM - 1))
            ctxn = ap1.tile([P, NPC], f32)
            for h in range(H):
                rden = ap2.tile([1, NPC], f32, tag="rden")
                nc.vector.reciprocal(out=rden[:], in_=ctx_ps[h][32:33, :])
                bc = aps4.tile([P, NPC], f32, tag="scr")
                nc.tensor.matmul(out=bc[0:32, :], lhsT=ones_row[:], rhs=rden[:],
                                 start=True, stop=True)
                bc_sb = ap2.tile([32, NPC], f32, tag="bcsb")
                act_copy(out=bc_sb[:], in_=bc[0:32, :])
                nc.vector.tensor_tensor(
                    out=ctxn[DH * h:DH * (h + 1), :], in0=ctx_ps[h][0:32, :],
                    in1=bc_sb[:], op=ALU.mult)
            for tn in range(NNB):
                ops = aps4.tile([P, NPC], f32, tag="scr")
                nc.tensor.matmul(out=ops[:, 0:D],
                                 lhsT=ctxn[:, tn * P:(tn + 1) * P],
                                 rhs=wsb["wo"][:], start=True, stop=True)
                on_sb = ap2.tile([P, D], f32, tag="onsb")
                act_copy(out=on_sb[:], in_=ops[:, 0:D])
                if flags["bo"]:
                    nc.vector.tensor_tensor(out=on_sb[:], in0=on_sb[:],
                                            in1=wsb["bt_bo"][:], op=ALU.add)
                nc.sync.dma_start(out=o_node[tn * P:(tn + 1) * P, :],
                                  in_=on_sb[:])

    nc.compile()
    return nc


# ---------------------------------------------------------------------------
# driver
# ---------------------------------------------------------------------------

LAST = None


def _in_maps(st, cores, w):
    maps = []
    for c in range(st["C"]):
        cc = cores[c]
        m = dict(
            lef_s=cc["lef_s"], ef_s=cc["ef_s"], nf_s=cc["nf_s"],
            ip1=cc["ip1"], ip2=cc["ip2"], ldc=cc["ldc"], invc=cc["invc"],
            ieh=cc["ieh"], dstc=cc["dstc"], invd=cc["invd"],
        )
        for k, v in w.items():
            m["w_" + k] = v
        maps.append(m)
    return maps


def _unshard(st, cores, outs):
    L, E, NN = st["L"], st["E"], st["NN"]
    line = np.empty((L, D), np.float32)
    edge = np.empty((E, D), np.float32)
    node = np.empty((NN, D), np.float32)
    for c in range(st["C"]):
        cc = cores[c]
        lid = cc["lid"]
        mask = lid >= 0
        line[lid[mask]] = outs[c]["out_line"][mask]
        edge[cc["eids"]] = outs[c]["out_edge"]
        node[cc["nids"]] = outs[c]["out_node"]
    return node, edge, line


def run(inputs, n_cores=8, use_sim=False, silu_native=None, slice_rows=32768):
    global LAST
    st, cores = _prep(inputs, n_cores, slice_rows=slice_rows)
    w, flags = _fold_weights(inputs, st["G"])
    if silu_native is None:
        silu_native = not use_sim
    nc = _build(st, (w, flags), silu_native=silu_native)
    maps = _in_maps(st, cores, w)

    if use_sim:
        import concourse.bass_interp as bass_interp
        sim = bass_interp.MultiCoreSim(nc, n_cores)
        for c in range(n_cores):
            for k, v in maps[c].items():
                sim.cores[c].tensor(k)[:] = v
        sim.simulate(check_with_hw=False)
        outs = [{k: np.array(sim.cores[c].mem_tensor(k))
                 for k in ("out_line", "out_edge", "out_node")}
                for c in range(n_cores)]
    else:
        from concourse.bass_utils import run_bass_kernel_spmd
        import os
        trace = bool(int(os.environ.get("KERNEL_TRACE", "0")))
        res = run_bass_kernel_spmd(nc, maps, core_ids=list(range(n_cores)),
                                   trace=trace)
        LAST = res
        outs = res.results
    return _unshard(st, cores, outs)


def kernel(**inputs):
    node, edge, line = run(inputs, n_cores=8, use_sim=False)
    return node, edge, line


# revision 18
# speedup vs baseline: 1.2144x; 1.2144x over previous
"""ALIGNN layer on 8 Trainium2 NeuronCores (Bass/Tile).

Strategy:
- Edges balance-packed into 128-edge blocks per core (by line-edge count).
- Lines owned by their ld-edge's core, grouped per 4-block group, bucket-
  ordered by P1-slice (32768-row windows) for int16 dma_gather.
- line MLP1 factored: concat(ef[ls],ef[ld])@W1 = (ef@W1a)[ls] + (ef@W1b)[ld]
  -> dense P1/P2 per shard, AllGather P1, dma_gather per slice.
- h_pre transposed+accumulated on TensorE (P1.T + P2.T into PSUM), silu is
  the PSUM->SBUF move, mlp2 consumes feat-major hT as lhsT -> row-major y.
- scatter-mean = wide is_equal selector matmuls on TensorE + 1/cnt scale.
- Global attention computed transposed; softmax denom via ones-column in V.
"""

import math
import numpy as np

D = 128
H = 4
DH = 32
P = 128


def _snake_pack(weights, n_bins):
    n = len(weights)
    assert n % n_bins == 0
    order = np.argsort(-weights, kind="stable")
    bins = np.empty(n, dtype=np.int64)
    fwd = np.arange(n_bins)
    rev = fwd[::-1]
    for r in range(n // n_bins):
        sel = order[r * n_bins:(r + 1) * n_bins]
        bins[sel] = fwd if (r % 2 == 0) else rev
    return bins


def _wrap_idx(flat, cap):
    """[cap] -> [128, cap//16] int16 device layout (wrapped in 16 partitions,
    replicated for the 8 Q7 cores)."""
    assert cap % 16 == 0
    w = flat.reshape(cap // 16, 16).T.astype(np.int16)   # [16, cap//16]
    return np.tile(w, (8, 1))


def _prep(inputs, C, slice_rows=32768):
    nf = np.asarray(inputs["node_features"], np.float32)
    ef = np.asarray(inputs["edge_features"], np.float32)
    lef = np.asarray(inputs["line_edge_features"], np.float32)
    ei = np.asarray(inputs["edge_index"], np.int64)
    lei = np.asarray(inputs["line_edge_index"], np.int64)

    NN, _ = nf.shape
    E, _ = ef.shape
    L, _ = lef.shape
    EPC = E // C
    NPC = NN // C
    NBLK = EPC // P
    NNB = NPC // P
    assert E % (C * P) == 0 and NN % (C * P) == 0

    ls, ld = lei[0], lei[1]
    dst = ei[1]

    # ---- edge blocks (balanced by line count) ----
    cnt = np.bincount(ld, minlength=E).astype(np.int64)
    eblk = _snake_pack(cnt, C * NBLK)
    srt = np.argsort(eblk, kind="stable")
    pos_in_sorted = np.empty(E, dtype=np.int64)
    pos_in_sorted[srt] = np.arange(E)
    blk_start = np.searchsorted(eblk[srt], np.arange(C * NBLK))
    eslot = pos_in_sorted - blk_start[eblk]
    perm_pos = eblk * P + eslot
    edge_at = np.empty(E, dtype=np.int64)
    edge_at[perm_pos] = np.arange(E)

    # slices of the permuted edge table (for int16 gathers)
    NSLC = int(math.ceil(E / slice_rows))
    SLICE = int(math.ceil(E / NSLC / P)) * P
    NSLC = int(math.ceil(E / SLICE))
    assert SLICE <= 32768

    G = 4 if NBLK % 4 == 0 else (2 if NBLK % 2 == 0 else 1)
    NGRP = NBLK // G

    # ---- assign lines to (core, group, slice-bucket) ----
    lgrp = eblk[ld] // G                        # global group id of each line
    lslc = perm_pos[ls] // SLICE                # src slice of P1 row
    key = lgrp * NSLC + lslc
    lsrt = np.argsort(key, kind="stable")
    kb_start = np.searchsorted(key[lsrt], np.arange(C * NGRP * NSLC + 1))
    bucket_sizes = np.diff(kb_start)
    CAP = int(math.ceil(max(bucket_sizes.max(), 1) / P)) * P
    CAPC = CAP // P                             # chunks per bucket
    NCH = NSLC * CAPC                           # chunks per group
    LPG = NCH * P                               # line slots per group
    LPC = NGRP * LPG

    # ---- node blocks ----
    indeg = np.bincount(dst, minlength=NN).astype(np.int64)
    nblk = _snake_pack(indeg, C * NNB)
    nsrt0 = np.argsort(nblk, kind="stable")
    npos = np.empty(NN, dtype=np.int64)
    npos[nsrt0] = np.arange(NN)
    nb_start = np.searchsorted(nblk[nsrt0], np.arange(C * NNB))
    nslot = npos - nb_start[nblk]
    nperm_pos = nblk * P + nslot
    node_at = np.empty(NN, dtype=np.int64)
    node_at[nperm_pos] = np.arange(NN)

    # ---- edges grouped by (dst node block, slice of eh row) ----
    eslc = perm_pos // SLICE
    nkey = nblk[dst] * NSLC + eslc
    dsrt = np.argsort(nkey, kind="stable")
    nkb_start = np.searchsorted(nkey[dsrt], np.arange(C * NNB * NSLC + 1))
    nbucket = np.diff(nkb_start)
    CAPN = int(math.ceil(max(nbucket.max(), 1) / P)) * P
    CAPNC = CAPN // P
    KCNE = NSLC * CAPNC                         # chunks per node block

    st = dict(C=C, NN=NN, E=E, L=L, EPC=EPC, NPC=NPC, NBLK=NBLK, NNB=NNB,
              G=G, NGRP=NGRP, NSLC=NSLC, SLICE=SLICE, CAP=CAP, CAPC=CAPC,
              NCH=NCH, LPG=LPG, LPC=LPC, CAPN=CAPN, CAPNC=CAPNC, KCNE=KCNE)

    cores = []
    for c in range(C):
        # ----- line slots -----
        lid = np.full(LPC, -1, dtype=np.int64)
        ip1 = np.zeros((NGRP, NSLC, CAP), np.int64)
        for g in range(NGRP):
            for s in range(NSLC):
                kb = (c * NGRP + g) * NSLC + s
                ids = lsrt[kb_start[kb]:kb_start[kb + 1]]
                base = g * LPG + s * CAP
                lid[base:base + len(ids)] = ids
                ip1[g, s, :len(ids)] = perm_pos[ls[ids]] - s * SLICE
        pad = lid < 0
        lid0 = np.where(pad, 0, lid)
        lef_s = lef[lid0].copy()
        lef_s[pad] = 0.0
        ip2 = (perm_pos[ld[lid0]] - c * EPC)
        ip2[pad] = 0
        ldc = (eblk[ld[lid0]] % G) * P + eslot[ld[lid0]]
        ldc = ldc.astype(np.float32)
        ldc[pad] = -1.0

        ip1_d = np.zeros((NGRP, NSLC, P, CAP // 16), np.int16)
        for g in range(NGRP):
            for s in range(NSLC):
                ip1_d[g, s] = _wrap_idx(ip1[g, s], CAP)
        ip2_d = np.zeros((NGRP, P, LPG // 16), np.int16)
        for g in range(NGRP):
            ip2_d[g] = _wrap_idx(ip2[g * LPG:(g + 1) * LPG], LPG)
        ldc_d = np.ascontiguousarray(
            ldc.reshape(NGRP, NCH, P).transpose(0, 2, 1))     # [NGRP,128,NCH]

        cnt_c = cnt[edge_at[c * EPC:(c + 1) * EPC]].reshape(NBLK, P)
        invc = np.ascontiguousarray(
            (1.0 / np.clip(cnt_c, 1, None)).astype(np.float32).T)
        ef_s = ef[edge_at[c * EPC:(c + 1) * EPC]].copy()
        nf_s = nf[node_at[c * NPC:(c + 1) * NPC]].copy()

        # ----- node-stage edge slots -----
        eid_n = np.full(NNB * KCNE * P, -1, dtype=np.int64)
        iehs = np.zeros((NNB, NSLC, CAPN), np.int64)
        for nb in range(NNB):
            for s in range(NSLC):
                kb = (c * NNB + nb) * NSLC + s
                ids = dsrt[nkb_start[kb]:nkb_start[kb + 1]]
                base = nb * KCNE * P + s * CAPN
                eid_n[base:base + len(ids)] = ids
                iehs[nb, s, :len(ids)] = perm_pos[ids] - s * SLICE
        npad = eid_n < 0
        eid_n0 = np.where(npad, 0, eid_n)
        dstc = nslot[dst[eid_n0]].astype(np.float32)
        dstc[npad] = -1.0
        ieh_d = np.zeros((NNB, NSLC, P, CAPN // 16), np.int16)
        for nb in range(NNB):
            for s in range(NSLC):
                ieh_d[nb, s] = _wrap_idx(iehs[nb, s], CAPN)
        dstc_d = np.ascontiguousarray(
            dstc.reshape(NNB, KCNE, P).transpose(0, 2, 1))    # [NNB,128,KCNE]
        ind_c = indeg[node_at[c * NPC:(c + 1) * NPC]].reshape(NNB, P)
        invd = np.ascontiguousarray(
            (1.0 / np.clip(ind_c, 1, None)).astype(np.float32).T)

        cores.append(dict(
            lef_s=lef_s, ef_s=ef_s, nf_s=nf_s,
            ip1=ip1_d, ip2=ip2_d, ldc=ldc_d, invc=invc,
            ieh=ieh_d, dstc=dstc_d, invd=invd,
            lid=lid, eids=edge_at[c * EPC:(c + 1) * EPC],
            nids=node_at[c * NPC:(c + 1) * NPC],
        ))
    return st, cores


def _fold_weights(inputs, G):
    g = lambda k: np.asarray(inputs[k], np.float32)
    w = {}
    le_w1 = g("le_w1")
    w["w1cat"] = np.ascontiguousarray(
        np.concatenate([le_w1[:D], le_w1[D:]], axis=1))      # [128,256]
    w["lw2"] = g("le_w2")
    e_w1 = g("e_w1")
    w["ew1ef"] = np.ascontiguousarray(e_w1[:D] + e_w1[D:2 * D])   # [128,256]
    w["ew1la"] = np.ascontiguousarray(e_w1[2 * D:])
    w["ew2"] = np.ascontiguousarray(
        g("e_w2").reshape(2, D, D).transpose(1, 0, 2))        # [128,2,128]
    n_w1 = g("n_w1")
    w["nw1"] = np.ascontiguousarray(
        n_w1.reshape(2, D, 2 * D).transpose(1, 0, 2))         # [128,2,256]
    w["nw2"] = np.ascontiguousarray(
        g("n_w2").reshape(2, D, D).transpose(1, 0, 2))
    wqkv = g("wqkv")
    w["wq"] = np.ascontiguousarray(wqkv[:, :D] / math.sqrt(DH))
    w["wk"] = np.ascontiguousarray(wqkv[:, D:2 * D])
    w["wv"] = np.ascontiguousarray(wqkv[:, 2 * D:])
    w["wo"] = g("wo")
    w["iota"] = np.ascontiguousarray(
        np.broadcast_to(np.arange(G * P, dtype=np.float32), (P, G * P)))

    flags = {}
    def nz(k):
        return not np.allclose(g(k), 0.0)
    def row_tile(k):
        v = g(k)
        return np.ascontiguousarray(
            np.broadcast_to(v[None, :], (P, v.shape[0])).astype(np.float32))
    for k in ["le_b1", "le_b2", "e_b1", "e_b2", "n_b1", "n_b2", "bqkv", "bo"]:
        flags[k] = nz(k)
    if flags["le_b1"]:
        w["lb1_col"] = np.ascontiguousarray(g("le_b1").reshape(D, 1))
    if flags["le_b2"]:
        w["bt_le_b2"] = row_tile("le_b2")
    if flags["e_b1"]:
        w["eb1_col"] = np.ascontiguousarray(g("e_b1").reshape(2, D).T.copy())
    if flags["e_b2"]:
        w["bt_e_b2"] = row_tile("e_b2")
    if flags["n_b1"]:
        w["nb1_col"] = np.ascontiguousarray(g("n_b1").reshape(2, D).T.copy())
    if flags["n_b2"]:
        w["bt_n_b2"] = row_tile("n_b2")
    if flags["bo"]:
        w["bt_bo"] = row_tile("bo")
    if flags["bqkv"]:
        bq = g("bqkv")
        w["bq_col"] = np.ascontiguousarray(
            (bq[:D] / math.sqrt(DH)).reshape(D, 1))
        w["bk_col"] = np.ascontiguousarray(bq[D:2 * D].reshape(D, 1))
        w["bv_row"] = np.ascontiguousarray(
            np.broadcast_to(bq[None, 2 * D:], (P, D)).astype(np.float32))
    lngs = {}
    for nm in ["node", "edge", "line"]:
        gv, bv = g(f"ln_{nm}_g"), g(f"ln_{nm}_b")
        triv = np.allclose(gv, 1.0) and np.allclose(bv, 0.0)
        lngs[nm] = not triv
        if not triv:
            w[f"lng_{nm}"] = np.ascontiguousarray(
                np.broadcast_to(gv[None, :], (P, D)).astype(np.float32))
            w[f"lnb_{nm}"] = np.ascontiguousarray(
                np.broadcast_to(bv[None, :], (P, D)).astype(np.float32))
    flags["ln"] = lngs
    return w, flags


# ---------------------------------------------------------------------------
# bass builder
# ---------------------------------------------------------------------------

def _build(st, wflags, silu_native=True):
    import concourse.bass as bass
    import concourse.bacc as bacc
    import concourse.tile as tile
    from concourse import mybir
    from concourse.masks import make_identity
    from contextlib import ExitStack

    w, flags = wflags
    f32 = mybir.dt.float32
    bf = mybir.dt.bfloat16
    i32 = mybir.dt.int32
    i16 = mybir.dt.int16
    AF = mybir.ActivationFunctionType
    ALU = mybir.AluOpType

    C, EPC, NPC = st["C"], st["EPC"], st["NPC"]
    NBLK, NNB = st["NBLK"], st["NNB"]
    G, NGRP, NSLC, SLICE = st["G"], st["NGRP"], st["NSLC"], st["SLICE"]
    CAP, CAPC, NCH, LPG, LPC = (st["CAP"], st["CAPC"], st["NCH"],
                                st["LPG"], st["LPC"])
    CAPN, CAPNC, KCNE = st["CAPN"], st["CAPNC"], st["KCNE"]
    NN = st["NN"]
    NTM = NN // P
    RG = [list(range(C))]
    E_PERM = NBLK * P * C

    nc = bacc.Bacc(None, num_devices=C)

    def dp(name, shape, dt=f32, out=False):
        return nc.declare_dram_parameter(name, list(shape), dt, isOutput=out)

    t_lef = dp("lef_s", (LPC, D))
    t_ef = dp("ef_s", (EPC, D))
    t_nf = dp("nf_s", (NPC, D))
    t_ip1 = dp("ip1", (NGRP, NSLC, P, CAP // 16), i16)
    t_ip2 = dp("ip2", (NGRP, P, LPG // 16), i16)
    t_ldc = dp("ldc", (NGRP, P, NCH))
    t_invc = dp("invc", (P, NBLK))
    t_ieh = dp("ieh", (NNB, NSLC, P, CAPN // 16), i16)
    t_dstc = dp("dstc", (NNB, P, KCNE))
    t_invd = dp("invd", (P, NNB))
    wt = {}
    for k, v in w.items():
        wt[k] = dp("w_" + k, v.shape)
    o_line = dp("out_line", (LPC, D), out=True)
    o_edge = dp("out_edge", (EPC, D), out=True)
    o_node = dp("out_node", (NPC, D), out=True)

    with tile.TileContext(nc) as tc, ExitStack() as ctx:
        dram = ctx.enter_context(tc.tile_pool(name="dram", bufs=1, space="DRAM"))
        SHR = "Shared" if C > 4 else "Local"
        p1_loc = dram.tile([EPC, D], bf)
        p2_loc = dram.tile([EPC, D], bf)
        p1_full = dram.tile([E_PERM, D], bf, addr_space=SHR)
        eh_loc = dram.tile([EPC, D], bf)
        eh_full = dram.tile([E_PERM, D], bf, addr_space=SHR)
        kt_loc = dram.tile([P, NPC], f32)
        kt_full = dram.tile([C, P, NPC], f32, addr_space=SHR)
        v_loc = dram.tile([NPC, D], f32)
        v_full = dram.tile([NN, D], f32, addr_space=SHR)

        cst = ctx.enter_context(tc.tile_pool(name="cst", bufs=1))
        ident = cst.tile([P, P], f32)
        make_identity(nc, ident[:])
        ident_b = cst.tile([P, P], bf)
        make_identity(nc, ident_b[:])
        iota_t = cst.tile([P, G * P], f32)
        nc.sync.dma_start(out=iota_t[:], in_=wt["iota"][:, :])
        eps_t = cst.tile([P, 1], f32)
        nc.vector.memset(eps_t[:], 1e-5)
        ones_row = cst.tile([1, 32], f32)
        nc.vector.memset(ones_row[:], 1.0)

        BF_W = {"w1cat", "lw2", "ew1ef", "ew1la", "ew2", "nw1", "nw2"}
        wsb = {}
        for k, v in w.items():
            if k == "iota":
                continue
            dt_k = bf if k in BF_W else f32
            wsb[k] = cst.tile(list(v.shape), dt_k, name="w_" + k)
            if dt_k == bf:
                nc.gpsimd.dma_start(out=wsb[k][:], in_=wt[k][:])
            else:
                nc.sync.dma_start(out=wsb[k][:], in_=wt[k][:])
        invc_sb = cst.tile([P, NBLK], f32)
        nc.sync.dma_start(out=invc_sb[:], in_=t_invc[:, :])
        invd_sb = cst.tile([P, NNB], f32)
        nc.sync.dma_start(out=invd_sb[:], in_=t_invd[:, :])

        efT = cst.tile([P, EPC], bf)

        def silu_op(out_ap, in_ap, tmp_pool, shape, tag, bias=None):
            if silu_native:
                if bias is None:
                    nc.scalar.activation(out=out_ap, in_=in_ap, func=AF.Silu)
                else:
                    nc.scalar.activation(out=out_ap, in_=in_ap, func=AF.Silu,
                                         bias=bias, scale=1.0)
            else:
                src = in_ap
                if bias is not None:
                    b = tmp_pool.tile(shape, f32, tag=tag + "b",
                                      name="sb_" + tag)
                    nc.vector.tensor_scalar(out=b[:], in0=in_ap, scalar1=bias,
                                            scalar2=None, op0=ALU.add)
                    src = b[:]
                tmp = tmp_pool.tile(shape, f32, tag=tag, name="silu_" + tag)
                nc.scalar.activation(out=tmp[:], in_=src, func=AF.Sigmoid)
                nc.vector.tensor_tensor(out=out_ap, in0=src, in1=tmp[:],
                                        op=ALU.mult)

        dve_copy = nc.vector.tensor_copy
        act_copy = nc.scalar.copy

        def rsqrt_dve(pool, out_ap, var_ap, shape, tag):
            """out = 1/sqrt(var + eps) using int bit-trick + 2 Newton iters,
            DVE only (avoids ACT table-set ping-pong with Silu)."""
            ve = pool.tile(shape, f32, tag=tag + "ve", name="rs_ve" + tag)
            nc.vector.tensor_scalar(out=ve[:], in0=var_ap, scalar1=1e-5,
                                    scalar2=None, op0=ALU.add)
            # seed: r = bitcast(0x5f3759df - (bits(v) >> 1))
            nc.vector.tensor_scalar(
                out=out_ap.bitcast(i32), in0=ve[:].bitcast(i32),
                scalar1=1, scalar2=None, op0=ALU.arith_shift_right)
            nc.vector.tensor_scalar(
                out=out_ap.bitcast(i32), in0=out_ap.bitcast(i32),
                scalar1=0x5f3759df, scalar2=-1, op0=ALU.subtract, op1=ALU.mult)
            t = pool.tile(shape, f32, tag=tag + "t", name="rs_t" + tag)
            for _ in range(2):
                nc.vector.tensor_tensor(out=t[:], in0=out_ap, in1=out_ap,
                                        op=ALU.mult)
                nc.vector.tensor_tensor(out=t[:], in0=t[:], in1=ve[:],
                                        op=ALU.mult)
                nc.vector.tensor_scalar(out=t[:], in0=t[:], scalar1=-0.5,
                                        scalar2=1.5, op0=ALU.mult, op1=ALU.add)
                nc.vector.tensor_tensor(out=out_ap, in0=out_ap, in1=t[:],
                                        op=ALU.mult)

        def tpose(pool_ps, dst_ap, src_ap, copy_eng, tag="tp"):
            ps = pool_ps.tile([P, P], f32, tag=tag)
            nc.tensor.transpose(out=ps[:], in_=src_ap, identity=ident[:])
            copy_eng(out=dst_ap, in_=ps[:])

        # =============== phase D: dense P1/P2 + efT ===============
        with tc.tile_pool(name="dph", bufs=3) as dph, \
             tc.tile_pool(name="dps", bufs=2, space="PSUM") as dps:
            for g in range(NBLK // G):
                ef_t = dph.tile([P, G, D], f32, tag="ef")
                nc.sync.dma_start(
                    out=ef_t[:],
                    in_=t_ef[g * G * P:(g + 1) * G * P, :].rearrange(
                        "(q p) f -> p q f", p=P))
                pq_sb = dph.tile([P, G, 2 * D], bf, tag="pq")
                for t in range(G):
                    b = g * G + t
                    tpose(dps, efT[:, b * P:(b + 1) * P], ef_t[:, t, :], dve_copy)
                    ps = dps.tile([P, 2 * D], f32, tag="mm")
                    nc.tensor.matmul(out=ps[:], lhsT=efT[:, b * P:(b + 1) * P],
                                     rhs=wsb["w1cat"][:], start=True, stop=True)
                    act_copy(out=pq_sb[:, t, :], in_=ps[:])
                nc.sync.dma_start(
                    out=p1_loc[g * G * P:(g + 1) * G * P, :].rearrange(
                        "(q p) f -> p q f", p=P),
                    in_=pq_sb[:, :, 0:D])
                nc.sync.dma_start(
                    out=p2_loc[g * G * P:(g + 1) * G * P, :].rearrange(
                        "(q p) f -> p q f", p=P),
                    in_=pq_sb[:, :, D:2 * D])

        nc.gpsimd.collective_compute(
            "AllGather", ALU.bypass, replica_groups=RG,
            ins=[p1_loc[:, :].opt()], outs=[p1_full[:, :].opt()])

        # =============== line + edge stage ===============
        SUB = 6 if NCH % 6 == 0 else (4 if NCH % 4 == 0 else
                                      (3 if NCH % 3 == 0 else
                                       (2 if NCH % 2 == 0 else 1)))
        NSUB = NCH // SUB
        with tc.tile_pool(name="lp", bufs=2) as lp, \
             tc.tile_pool(name="lp3", bufs=3) as lp3, \
             tc.tile_pool(name="lps", bufs=2, space="PSUM") as lps, \
             tc.tile_pool(name="lps1", bufs=1, space="PSUM") as lps1:
            for g in range(NGRP):
                idx1 = lp3.tile([P, NSLC, CAP // 16], i16, tag="i1")
                nc.sync.dma_start(out=idx1[:], in_=t_ip1[g].rearrange(
                    "s p c -> p s c"))
                idx2 = lp3.tile([P, LPG // 16], i16, tag="i2")
                nc.sync.dma_start(out=idx2[:], in_=t_ip2[g, :, :])
                ldc = lp.tile([P, NCH], f32, tag="ldc")
                nc.sync.dma_start(out=ldc[:], in_=t_ldc[g, :, :])
                lef_t = lp.tile([P, NCH, D], f32, tag="lef")
                nc.sync.dma_start(
                    out=lef_t[:],
                    in_=t_lef[g * LPG:(g + 1) * LPG, :].rearrange(
                        "(q p) f -> p q f", p=P))
                ef_row = lp.tile([P, G, D], f32, tag="efr")
                nc.sync.dma_start(
                    out=ef_row[:],
                    in_=t_ef[g * G * P:(g + 1) * G * P, :].rearrange(
                        "(q p) f -> p q f", p=P))

                hp1 = lp.tile([P, NCH, D], bf, tag="hp1")
                for s in range(NSLC):
                    nc.gpsimd.dma_gather(
                        out_ap=hp1[:, s * CAPC:(s + 1) * CAPC, :],
                        in_ap=p1_full[s * SLICE:min((s + 1) * SLICE, E_PERM), :],
                        idxs_ap=idx1[:, s, :], num_idxs=CAP, num_idxs_reg=CAP,
                        elem_size=D, single_packet=False)
                hp2 = lp.tile([P, NCH, D], bf, tag="hp2")
                nc.gpsimd.dma_gather(
                    out_ap=hp2[:], in_ap=p2_loc[:, :], idxs_ap=idx2[:],
                    num_idxs=LPG, num_idxs_reg=LPG, elem_size=D,
                    single_packet=False)

                nc.vector.tensor_tensor(out=hp1[:], in0=hp1[:], in1=hp2[:],
                                        op=ALU.add)
                vh = lp.tile([P, NCH, D], bf, tag="vh")
                yb = lp.tile([P, NCH, D], f32, tag="yb")
                mvs = lp.tile([P, NCH, 2], f32, tag="mvs")
                agg = lps1.tile([P, G, P], f32, tag="agg")
                eh_sb = lp.tile([P, G, D], f32, tag="ehsb")
                emv = lp.tile([P, G, 2], f32, tag="emv")
                for sub in range(NSUB):
                    for k in range(SUB):
                        q = sub * SUB + k
                        hT = lps.tile([P, P], bf, tag="tp")
                        nc.tensor.matmul(out=hT[:], lhsT=hp1[:, q, :],
                                         rhs=ident_b[:], is_transpose=True,
                                         start=True, stop=True)
                        hT_sb = lp3.tile([P, P], bf, tag="hts")
                        silu_op(hT_sb[:], hT[:], lp3, [P, P], "sl1",
                                bias=(wsb["lb1_col"][:] if flags["le_b1"]
                                      else None))
                        yk = lps.tile([P, P], f32, tag="yk")
                        nc.tensor.matmul(out=yk[:], lhsT=hT_sb[:],
                                         rhs=wsb["lw2"][:], start=True,
                                         stop=False)
                        nc.tensor.matmul(out=yk[:], lhsT=ident[:],
                                         rhs=lef_t[:, q, :], start=False,
                                         stop=True)
                        act_copy(out=yb[:, q, :], in_=yk[:])
                        if flags["le_b2"]:
                            nc.vector.tensor_tensor(
                                out=yb[:, q, :], in0=yb[:, q, :],
                                in1=wsb["bt_le_b2"][:], op=ALU.add)
                        stt = lp3.tile([P, 6], f32, tag="stt")
                        nc.vector.bn_stats(out=stt[:], in_=yb[:, q, :])
                        nc.vector.bn_aggr(out=mvs[:, q, :], in_=stt[:])
                    rstd = lp3.tile([P, SUB], f32, tag="rstd")
                    rsqrt_dve(lp3, rstd[:], mvs[:, sub * SUB:(sub + 1) * SUB, 1],
                              [P, SUB], "lr")
                    for k in range(SUB):
                        q = sub * SUB + k
                        nc.vector.tensor_scalar(
                            out=vh[:, q, :], in0=yb[:, q, :],
                            scalar1=mvs[:, q, 0:1], scalar2=rstd[:, k:k + 1],
                            op0=ALU.subtract, op1=ALU.mult)
                        if flags["ln"]["line"]:
                            nc.vector.tensor_tensor(
                                out=vh[:, q, :], in0=vh[:, q, :],
                                in1=wsb["lng_line"][:], op=ALU.mult)
                            nc.vector.tensor_tensor(
                                out=vh[:, q, :], in0=vh[:, q, :],
                                in1=wsb["lnb_line"][:], op=ALU.add)
                        sel = lp3.tile([P, G * P], bf, tag="sel")
                        nc.vector.tensor_scalar(
                            out=sel[:], in0=iota_t[:], scalar1=ldc[:, q:q + 1],
                            scalar2=None, op0=ALU.is_equal)
                        for b in range(G):
                            nc.tensor.matmul(
                                out=agg[:, b, :],
                                lhsT=sel[:, b * P:(b + 1) * P],
                                rhs=vh[:, q, :],
                                start=(q == 0 and b == 0),
                                stop=(q == NCH - 1 and b == G - 1))
                # write V-hat (bf16 -> f32 cast during DMA, SWDGE)
                nc.gpsimd.dma_start(
                    out=o_line[g * LPG:(g + 1) * LPG, :].rearrange(
                        "(q p) f -> p q f", p=P),
                    in_=vh[:])
                # ---- edge update for the G blocks ----
                for t in range(G):
                    b = g * G + t
                    la = lp3.tile([P, D], bf, tag="la")
                    nc.vector.tensor_scalar(
                        out=la[:], in0=agg[:, t, :], scalar1=invc_sb[:, b:b + 1],
                        scalar2=None, op0=ALU.mult)
                    laT = lp3.tile([P, D], bf, tag="laT")
                    laps = lps.tile([P, P], bf, tag="tp")
                    nc.tensor.matmul(out=laps[:], lhsT=la[:], rhs=ident_b[:],
                                     is_transpose=True, start=True, stop=True)
                    dve_copy(out=laT[:], in_=laps[:])
                    heT = lps.tile([P, 2, D], f32, tag="heT")
                    for hh in range(2):
                        nc.tensor.matmul(
                            out=heT[:, hh, :],
                            lhsT=wsb["ew1ef"][:, hh * P:(hh + 1) * P],
                            rhs=efT[:, b * P:(b + 1) * P],
                            start=(hh == 0), stop=False)
                        nc.tensor.matmul(
                            out=heT[:, hh, :],
                            lhsT=wsb["ew1la"][:, hh * P:(hh + 1) * P],
                            rhs=laT[:], start=False, stop=(hh == 1))
                    heT_sb = lp3.tile([P, 2, D], bf, tag="hesb")
                    for hh in range(2):
                        silu_op(heT_sb[:, hh, :], heT[:, hh, :], lp3,
                                [P, D], "sl2",
                                bias=(wsb["eb1_col"][:, hh:hh + 1]
                                      if flags["e_b1"] else None))
                    ye = lps.tile([P, D], f32, tag="yk")
                    nc.tensor.matmul(out=ye[:], lhsT=heT_sb[:, 0, :],
                                     rhs=wsb["ew2"][:, 0, :], start=True,
                                     stop=False)
                    nc.tensor.matmul(out=ye[:], lhsT=heT_sb[:, 1, :],
                                     rhs=wsb["ew2"][:, 1, :], start=False,
                                     stop=False)
                    nc.tensor.matmul(out=ye[:], lhsT=ident[:],
                                     rhs=ef_row[:, t, :], start=False,
                                     stop=True)
                    act_copy(out=eh_sb[:, t, :], in_=ye[:])
                    if flags["e_b2"]:
                        nc.vector.tensor_tensor(
                            out=eh_sb[:, t, :], in0=eh_sb[:, t, :],
                            in1=wsb["bt_e_b2"][:], op=ALU.add)
                    stt2 = lp3.tile([P, 6], f32, tag="stt2")
                    nc.vector.bn_stats(out=stt2[:], in_=eh_sb[:, t, :])
                    nc.vector.bn_aggr(out=emv[:, t, :], in_=stt2[:])
                erstd = lp3.tile([P, G], f32, tag="erstd")
                rsqrt_dve(lp3, erstd[:], emv[:, :, 1], [P, G], "er")
                ehn = lp.tile([P, G, D], f32, tag="ehn")
                for t in range(G):
                    nc.vector.tensor_scalar(
                        out=ehn[:, t, :], in0=eh_sb[:, t, :],
                        scalar1=emv[:, t, 0:1], scalar2=erstd[:, t:t + 1],
                        op0=ALU.subtract, op1=ALU.mult)
                    if flags["ln"]["edge"]:
                        nc.vector.tensor_tensor(
                            out=ehn[:, t, :], in0=ehn[:, t, :],
                            in1=wsb["lng_edge"][:], op=ALU.mult)
                        nc.vector.tensor_tensor(
                            out=ehn[:, t, :], in0=ehn[:, t, :],
                            in1=wsb["lnb_edge"][:], op=ALU.add)
                nc.sync.dma_start(
                    out=o_edge[g * G * P:(g + 1) * G * P, :].rearrange(
                        "(q p) f -> p q f", p=P),
                    in_=ehn[:])
                nc.gpsimd.dma_start(
                    out=eh_loc[g * G * P:(g + 1) * G * P, :].rearrange(
                        "(q p) f -> p q f", p=P),
                    in_=ehn[:])

        nc.gpsimd.collective_compute(
            "AllGather", ALU.bypass, replica_groups=RG,
            ins=[eh_loc[:, :].opt()], outs=[eh_full[:, :].opt()])

        # =============== node stage ===============
        xh = cst.tile([P, NNB, D], f32)
        with tc.tile_pool(name="np2", bufs=2) as np2, \
             tc.tile_pool(name="nps", bufs=2, space="PSUM") as nps, \
             tc.tile_pool(name="nps1", bufs=1, space="PSUM") as nps1:
            for nb in range(NNB):
                idxn = np2.tile([P, NSLC, CAPN // 16], i16, tag="ixn")
                nc.sync.dma_start(out=idxn[:], in_=t_ieh[nb].rearrange(
                    "s p c -> p s c"))
                dstc = np2.tile([P, KCNE], f32, tag="dstc")
                nc.sync.dma_start(out=dstc[:], in_=t_dstc[nb, :, :])
                gath = np2.tile([P, KCNE, D], bf, tag="gath")
                for s in range(NSLC):
                    nc.gpsimd.dma_gather(
                        out_ap=gath[:, s * CAPNC:(s + 1) * CAPNC, :],
                        in_ap=eh_full[s * SLICE:min((s + 1) * SLICE, E_PERM), :],
                        idxs_ap=idxn[:, s, :], num_idxs=CAPN, num_idxs_reg=CAPN,
                        elem_size=D, single_packet=False)
                nagg = nps1.tile([P, D], f32, tag="nagg")
                for k in range(KCNE):
                    sel = np2.tile([P, P], bf, tag="nsel")
                    nc.vector.tensor_scalar(
                        out=sel[:], in0=iota_t[:, 0:P], scalar1=dstc[:, k:k + 1],
                        scalar2=None, op0=ALU.is_equal)
                    nc.tensor.matmul(out=nagg[:], lhsT=sel[:], rhs=gath[:, k, :],
                                     start=(k == 0), stop=(k == KCNE - 1))
                na = np2.tile([P, D], bf, tag="na")
                nc.vector.tensor_scalar(
                    out=na[:], in0=nagg[:], scalar1=invd_sb[:, nb:nb + 1],
                    scalar2=None, op0=ALU.mult)
                nfr = np2.tile([P, D], f32, tag="nfr")
                nc.sync.dma_start(out=nfr[:], in_=t_nf[nb * P:(nb + 1) * P, :])
                nfT = np2.tile([P, D], bf, tag="nfT")
                tpose(nps, nfT[:], nfr[:], dve_copy)
                naT = np2.tile([P, D], bf, tag="naT")
                nps_b = nps.tile([P, P], bf, tag="tpb")
                nc.tensor.matmul(out=nps_b[:], lhsT=na[:], rhs=ident_b[:],
                                 is_transpose=True, start=True, stop=True)
                dve_copy(out=naT[:], in_=nps_b[:])
                hnT = nps1.tile([P, 2, D], f32, tag="hnT")
                for hh in range(2):
                    nc.tensor.matmul(
                        out=hnT[:, hh, :],
                        lhsT=wsb["nw1"][:, 0, hh * P:(hh + 1) * P],
                        rhs=nfT[:], start=(hh == 0), stop=False)
                    nc.tensor.matmul(
                        out=hnT[:, hh, :],
                        lhsT=wsb["nw1"][:, 1, hh * P:(hh + 1) * P],
                        rhs=naT[:], start=False, stop=(hh == 1))
                hnT_sb = np2.tile([P, 2, D], bf, tag="hnsb")
                for hh in range(2):
                    silu_op(hnT_sb[:, hh, :], hnT[:, hh, :], np2, [P, D], "sl3",
                            bias=(wsb["nb1_col"][:, hh:hh + 1]
                                  if flags["n_b1"] else None))
                yn = nps.tile([P, D], f32, tag="yn")
                nc.tensor.matmul(out=yn[:], lhsT=hnT_sb[:, 0, :],
                                 rhs=wsb["nw2"][:, 0, :], start=True, stop=False)
                nc.tensor.matmul(out=yn[:], lhsT=hnT_sb[:, 1, :],
                                 rhs=wsb["nw2"][:, 1, :], start=False, stop=False)
                nc.tensor.matmul(out=yn[:], lhsT=ident[:], rhs=nfr[:],
                                 start=False, stop=True)
                yn_sb = np2.tile([P, D], f32, tag="ynsb")
                act_copy(out=yn_sb[:], in_=yn[:])
                if flags["n_b2"]:
                    nc.vector.tensor_tensor(out=yn_sb[:], in0=yn_sb[:],
                                            in1=wsb["bt_n_b2"][:], op=ALU.add)
                stt3 = np2.tile([P, 6], f32, tag="stt3")
                nc.vector.bn_stats(out=stt3[:], in_=yn_sb[:])
                nmv = np2.tile([P, 2], f32, tag="nmv")
                nc.vector.bn_aggr(out=nmv[:], in_=stt3[:])
                nrstd = np2.tile([P, 1], f32, tag="nrstd")
                rsqrt_dve(np2, nrstd[:], nmv[:, 1:2], [P, 1], "nr")
                nc.vector.tensor_scalar(
                    out=xh[:, nb, :], in0=yn_sb[:],
                    scalar1=nmv[:, 0:1], scalar2=nrstd[:],
                    op0=ALU.subtract, op1=ALU.mult)
                if flags["ln"]["node"]:
                    nc.vector.tensor_tensor(out=xh[:, nb, :], in0=xh[:, nb, :],
                                            in1=wsb["lng_node"][:], op=ALU.mult)
                    nc.vector.tensor_tensor(out=xh[:, nb, :], in0=xh[:, nb, :],
                                            in1=wsb["lnb_node"][:], op=ALU.add)

        # =============== attention ===============
        with tc.tile_pool(name="ap1", bufs=1) as ap1, \
             tc.tile_pool(name="ap2", bufs=2) as ap2, \
             tc.tile_pool(name="aps", bufs=1, space="PSUM") as aps, \
             tc.tile_pool(name="aps4", bufs=4, space="PSUM") as aps4:
            xT = ap1.tile([P, NPC], f32)
            for nb in range(NNB):
                tpose(aps4, xT[:, nb * P:(nb + 1) * P], xh[:, nb, :], dve_copy,
                      tag="scr")
            qT = ap1.tile([P, NPC], f32)
            kT = ap1.tile([P, NPC], f32)
            qps = aps4.tile([P, NPC], f32, tag="scr")
            nc.tensor.matmul(out=qps[:], lhsT=wsb["wq"][:], rhs=xT[:],
                             start=True, stop=True)
            if flags["bqkv"]:
                nc.vector.tensor_scalar(out=qT[:], in0=qps[:],
                                        scalar1=wsb["bq_col"][:],
                                        scalar2=None, op0=ALU.add)
            else:
                dve_copy(out=qT[:], in_=qps[:])
            kps = aps4.tile([P, NPC], f32, tag="scr")
            nc.tensor.matmul(out=kps[:], lhsT=wsb["wk"][:], rhs=xT[:],
                             start=True, stop=True)
            if flags["bqkv"]:
                nc.vector.tensor_scalar(out=kT[:], in0=kps[:],
                                        scalar1=wsb["bk_col"][:],
                                        scalar2=None, op0=ALU.add)
            else:
                dve_copy(out=kT[:], in_=kps[:])
            nc.sync.dma_start(out=kt_loc[:, :], in_=kT[:])
            vrow = ap1.tile([P, NNB, D], f32)
            for nb in range(NNB):
                vps = aps4.tile([P, NPC], f32, tag="scr")
                nc.tensor.matmul(out=vps[:, 0:D],
                                 lhsT=xT[:, nb * P:(nb + 1) * P],
                                 rhs=wsb["wv"][:], start=True, stop=True)
                if flags["bqkv"]:
                    nc.vector.tensor_tensor(out=vrow[:, nb, :], in0=vps[:, 0:D],
                                            in1=wsb["bv_row"][:], op=ALU.add)
                else:
                    act_copy(out=vrow[:, nb, :], in_=vps[:, 0:D])
            nc.sync.dma_start(out=v_loc[:, :].rearrange("(q p) f -> p q f", p=P),
                              in_=vrow[:])
            nc.gpsimd.collective_compute(
                "AllGather", ALU.bypass, replica_groups=RG,
                ins=[kt_loc[:, :].opt()], outs=[kt_full[:, :, :].opt()])
            nc.gpsimd.collective_compute(
                "AllGather", ALU.bypass, replica_groups=RG,
                ins=[v_loc[:, :].opt()], outs=[v_full[:, :].opt()])

            KTs = ap1.tile([P, C, NPC], f32)
            nc.sync.dma_start(out=KTs[:], in_=kt_full[:, :, :].rearrange(
                "c d n -> d c n"))
            vsb = ap1.tile([P, NTM, H, 34], f32)
            nc.vector.memset(vsb[:], 0.0)
            v4 = v_full[:, :].rearrange("(j p) (h u) -> p j h u", p=P, h=H)
            for h in range(H):
                nc.sync.dma_start(out=vsb[:, :, h, 0:32], in_=v4[:, :, h, :])
            nc.vector.memset(vsb[:, :, :, 32:33], 1.0)

            KTf = KTs[:].rearrange("d c n -> d (c n)")
            ctx_ps = [aps.tile([33, NPC], f32, name=f"ctx{h}") for h in range(H)]
            for j in range(NTM):
                es = ap2.tile([P, H, NPC], f32, tag="es")
                for h in range(H):
                    sps = aps4.tile([P, NPC], f32, tag="scr")
                    nc.tensor.matmul(
                        out=sps[:],
                        lhsT=KTf[DH * h:DH * (h + 1), j * P:(j + 1) * P],
                        rhs=qT[DH * h:DH * (h + 1), :],
                        start=True, stop=True, tile_position=(DH * h, 0))
                    nc.scalar.activation(out=es[:, h, :], in_=sps[:], func=AF.Exp)
                for h in range(H):
                    nc.tensor.matmul(
                        out=ctx_ps[h][:], lhsT=vsb[:, j, h, 0:33],
                        rhs=es[:, h, :], start=(j == 0), stop=(j == NTM - 1))
            ctxn = ap1.tile([P, NPC], f32)
            for h in range(H):
                rden = ap2.tile([1, NPC], f32, tag="rden")
                nc.vector.reciprocal(out=rden[:], in_=ctx_ps[h][32:33, :])
                bc = aps4.tile([P, NPC], f32, tag="scr")
                nc.tensor.matmul(out=bc[0:32, :], lhsT=ones_row[:], rhs=rden[:],
                                 start=True, stop=True)
                bc_sb = ap2.tile([32, NPC], f32, tag="bcsb")
                act_copy(out=bc_sb[:], in_=bc[0:32, :])
                nc.vector.tensor_tensor(
                    out=ctxn[DH * h:DH * (h + 1), :], in0=ctx_ps[h][0:32, :],
                    in1=bc_sb[:], op=ALU.mult)
            for tn in range(NNB):
                ops = aps4.tile([P, NPC], f32, tag="scr")
                nc.tensor.matmul(out=ops[:, 0:D],
                                 lhsT=ctxn[:, tn * P:(tn + 1) * P],
                                 rhs=wsb["wo"][:], start=True, stop=True)
                on_sb = ap2.tile([P, D], f32, tag="onsb")
                act_copy(out=on_sb[:], in_=ops[:, 0:D])
                if flags["bo"]:
                    nc.vector.tensor_tensor(out=on_sb[:], in0=on_sb[:],
                                            in1=wsb["bt_bo"][:], op=ALU.add)
                nc.sync.dma_start(out=o_node[tn * P:(tn + 1) * P, :],
                                  in_=on_sb[:])

    nc.compile()
    return nc


# ---------------------------------------------------------------------------
# driver
# ---------------------------------------------------------------------------

LAST = None


def _in_maps(st, cores, w):
    maps = []
    for c in range(st["C"]):
        cc = cores[c]
        m = dict(
            lef_s=cc["lef_s"], ef_s=cc["ef_s"], nf_s=cc["nf_s"],
            ip1=cc["ip1"], ip2=cc["ip2"], ldc=cc["ldc"], invc=cc["invc"],
            ieh=cc["ieh"], dstc=cc["dstc"], invd=cc["invd"],
        )
        for k, v in w.items():
            m["w_" + k] = v
        maps.append(m)
    return maps


def _unshard(st, cores, outs):
    L, E, NN = st["L"], st["E"], st["NN"]
    line = np.empty((L, D), np.float32)
    edge = np.empty((E, D), np.float32)
    node = np.empty((NN, D), np.float32)
    for c in range(st["C"]):
        cc = cores[c]
        lid = cc["lid"]
        mask = lid >= 0
        line[lid[mask]] = outs[c]["out_line"][mask]
        edge[cc["eids"]] = outs[c]["out_edge"]
        node[cc["nids"]] = outs[c]["out_node"]
    return node, edge, line


def run(inputs, n_cores=8, use_sim=False, silu_native=None, slice_rows=32768):
    global LAST
    st, cores = _prep(inputs, n_cores, slice_rows=slice_rows)
    w, flags = _fold_weights(inputs, st["G"])
    if silu_native is None:
        silu_native = not use_sim
    nc = _build(st, (w, flags), silu_native=silu_native)
    maps = _in_maps(st, cores, w)

    if use_sim:
        import concourse.bass_interp as bass_interp
        sim = bass_interp.MultiCoreSim(nc, n_cores)
        for c in range(n_cores):
            for k, v in maps[c].items():
                sim.cores[c].tensor(k)[:] = v
        sim.simulate(check_with_hw=False)
        outs = [{k: np.array(sim.cores[c].mem_tensor(k))
                 for k in ("out_line", "out_edge", "out_node")}
                for c in range(n_cores)]
    else:
        from concourse.bass_utils import run_bass_kernel_spmd
        import os
        trace = bool(int(os.environ.get("KERNEL_TRACE", "0")))
        res = run_bass_kernel_spmd(nc, maps, core_ids=list(range(n_cores)),
                                   trace=trace)
        LAST = res
        outs = res.results
    return _unshard(st, cores, outs)


def kernel(**inputs):
    node, edge, line = run(inputs, n_cores=8, use_sim=False)
    return node, edge, line
